# revision 10
# baseline (speedup 1.0000x reference)
"""Trainium2 Bass kernel for nn_GATModel (2-layer GAT + mean-pool + MLP head).

Strategy (8 NeuronCores, SPMD):
  - Edges sorted by dst; each core owns a contiguous 6250-node range and all
    edges pointing into it (edge-balanced to ~1%).
  - Layer-1 zcat table ([z_h|1]*H | el | er rows, bf16) is built REPLICATED on
    every core from the full feature matrix with 391 small bf16 matmuls
    (superblocks of 8 blocks -> one DMA each, host-side rho row permutation),
    which removes the layer-1 AllGather and local table copy entirely.
  - Edge phase per 128-node window: per-chunk indirect-DMA gathers of src rows
    (128 rows/op is the only batched form the core SWDGE ucode supports),
    attention weights w = exp(leakyrelu(el_src + er_dst)) on DVE/ACT (er
    expanded node->edge via a one-hot maskT matmul; er_w fetched by one
    indirect row-gather per window), weighted one-hot masks via dual-op
    tensor_scalar, aggregation by PE matmuls accumulated in PSUM; softmax
    normalization folds into the epilogue via per-head ones columns.
  - Layer-2 table is AllGathered (25.6MB) then copied to a plain local DRAM
    buffer before gathering (indirect reads from the Shared collective buffer
    are much slower on HW).
  - Mean-pool by graph fused into layer-2 epilogue, AllReduce of the pooled
    [192,128] matrix, dense head on PE.
"""
import math
import os
from contextlib import ExitStack

import numpy as np
import ml_dtypes

import concourse.bacc as bacc
import concourse.bass as bass
import concourse.tile as tile
from concourse import mybir
from concourse.bass_utils import run_bass_kernel_spmd
from concourse.masks import make_identity

dt = mybir.dt

N_NODES = 50000
N_EDGES = 800000
N_GRAPHS = 128
NEG = 0.2
NC = 8
NPC = N_NODES // NC            # 6250 nodes per core
NWIN = math.ceil(NPC / 128)    # 49 windows per core
R1, R2 = 128, 256              # table row elems (bf16)
H = 3
D1, D2 = 32, 64
X1C, X2C = 96, 192             # x1/x2 feature cols
SB = 8                         # phase-A superblock
NB1 = math.ceil(N_NODES / 128)         # 391
NSB = math.ceil(NB1 / SB)              # 49
N1PAD = (NB1 // SB) * SB * 128 + (NB1 % SB) * 128  # 50048

_CACHE = {}


def _ceil(a, b):
    return (a + b - 1) // b


def build_program(cpws):
    """Build the SPMD bass program. cpws[w] = chunks (128 edges) of window w."""
    nc = bacc.Bacc("TRN2", target_bir_lowering=False, debug=False, num_devices=NC)
    cpws = tuple(cpws)
    CPWM = max(cpws)                 # max chunks per window (tile sizing)
    offs = [0]
    for c_ in cpws:
        offs.append(offs[-1] + c_)
    TCH = offs[-1]                   # total chunks per core

    # ---------------- I/O ----------------
    featTb_in = nc.dram_tensor("featTb", [11, N1PAD], dt.bfloat16, kind="ExternalInput").ap()
    wcat1 = nc.dram_tensor("wcat1", [11, R1], dt.bfloat16, kind="ExternalInput").ap()
    wcat2 = nc.dram_tensor("wcat2", [X1C + 1, R2], dt.float32, kind="ExternalInput").ap()
    srcc1_in = nc.dram_tensor("srcc1", [128, TCH], dt.int32, kind="ExternalInput").ap()
    srcc2_in = nc.dram_tensor("srcc2", [128, TCH], dt.int32, kind="ExternalInput").ap()
    dstw1_in = nc.dram_tensor("dstw1", [128, NWIN], dt.int32, kind="ExternalInput").ap()
    drelc_in = nc.dram_tensor("drelc", [128, TCH], dt.float32, kind="ExternalInput").ap()
    dstb_in = nc.dram_tensor("dstb", [128, TCH * 128], dt.bfloat16, kind="ExternalInput").ap()
    gidc_in = nc.dram_tensor("gidc", [128, NWIN], dt.float32, kind="ExternalInput").ap()
    invc_in = nc.dram_tensor("invc", [128, NWIN], dt.float32, kind="ExternalInput").ap()
    d1a_in = nc.dram_tensor("d1a", [128, 64], dt.float32, kind="ExternalInput").ap()
    d1b_in = nc.dram_tensor("d1b", [65, 64], dt.float32, kind="ExternalInput").ap()
    d2_in = nc.dram_tensor("d2", [65, 1], dt.float32, kind="ExternalInput").ap()
    out_ext = nc.dram_tensor("out", [N_GRAPHS, 1], dt.float32, kind="ExternalOutput").ap()

    rg = [list(range(NC))]

    with tile.TileContext(nc) as tc, ExitStack() as ctx:
        cst = ctx.enter_context(tc.tile_pool(name="cst", bufs=1))
        sb = ctx.enter_context(tc.tile_pool(name="sb", bufs=3))
        gp = ctx.enter_context(tc.tile_pool(name="gp", bufs=CPWM + 16))
        ps = ctx.enter_context(tc.tile_pool(name="ps", bufs=1, space="PSUM"))
        ps2 = ctx.enter_context(tc.tile_pool(name="ps2", bufs=2, space="PSUM"))
        dr = ctx.enter_context(tc.tile_pool(name="dr", bufs=1, space="DRAM"))

        # ---------------- constants ----------------
        ident = cst.tile([128, 128], dt.float32)
        make_identity(nc, ident[:])
        iota_i = cst.tile([128, 128], dt.int32)
        nc.gpsimd.iota(iota_i[:], pattern=[[1, 128]], base=0, channel_multiplier=0)
        iota_row = cst.tile([128, 128], dt.bfloat16)
        nc.vector.tensor_copy(iota_row[:], iota_i[:])
        iotac_i = cst.tile([128, 1], dt.int32)
        nc.gpsimd.iota(iotac_i[:], pattern=[[1, 1]], base=0, channel_multiplier=1)
        iota_col = cst.tile([128, 1], dt.float32)
        nc.vector.tensor_copy(iota_col[:], iotac_i[:])

        # resident inputs
        srcc1 = cst.tile([128, TCH], dt.int32)
        nc.sync.dma_start(out=srcc1[:], in_=srcc1_in)
        srcc2 = cst.tile([128, TCH], dt.int32)
        nc.sync.dma_start(out=srcc2[:], in_=srcc2_in)
        dstw1 = cst.tile([128, NWIN], dt.int32)
        nc.sync.dma_start(out=dstw1[:], in_=dstw1_in)
        drelc = cst.tile([128, TCH], dt.float32)
        nc.sync.dma_start(out=drelc[:], in_=drelc_in)
        gidc = cst.tile([128, NWIN], dt.float32)
        nc.sync.dma_start(out=gidc[:], in_=gidc_in)
        invc = cst.tile([128, NWIN], dt.float32)
        nc.sync.dma_start(out=invc[:], in_=invc_in)
        w1sb = cst.tile([11, R1], dt.bfloat16)
        nc.sync.dma_start(out=w1sb[:], in_=wcat1)
        w2sb = cst.tile([X1C + 1, R2], dt.float32)
        nc.sync.dma_start(out=w2sb[:], in_=wcat2)
        d1a = cst.tile([128, 64], dt.float32)
        nc.sync.dma_start(out=d1a[:], in_=d1a_in)
        d1b = cst.tile([65, 64], dt.float32)
        nc.sync.dma_start(out=d1b[:], in_=d1b_in)
        d2w = cst.tile([65, 1], dt.float32)
        nc.sync.dma_start(out=d2w[:], in_=d2_in)

        t1_full = dr.tile([N1PAD, R1], dt.bfloat16, name="t1_full")
        t2_shard = dr.tile([NPC, R2], dt.bfloat16)
        pool_loc = dr.tile([X2C, N_GRAPHS], dt.float32)
        table2_t = dr.tile([N_NODES, R2], dt.bfloat16, addr_space="Shared", name="table2_t")
        table2l_t = dr.tile([N_NODES, R2], dt.bfloat16, name="table2l_t")
        pool_red_t = dr.tile([X2C, N_GRAPHS], dt.float32, addr_space="Shared", name="pool_red_t")
        table1 = t1_full[:, :]
        table2 = table2l_t[:, :]
        pool_red = pool_red_t[:, :]

        # ------- phase A: replicated layer-1 table (no collective) -------
        with tc.tile_pool(name="ft", bufs=1) as ftp:
            ftsb = ftp.tile([11, N1PAD], dt.bfloat16)
            nc.sync.dma_start(out=ftsb[:], in_=featTb_in)
            for s in range(NSB):
                nj = min(NB1 - s * SB, SB)
                zb8 = sb.tile([128, SB, R1], dt.bfloat16, tag="zb8", name="zb8")
                for half in range(2):
                    j0 = half * 4
                    jn = min(nj - j0, 4)
                    if jn <= 0:
                        break
                    zpA = ps2.tile([128, 4, R1], dt.float32, tag="zcp", name="zpA")
                    for j in range(j0, j0 + jn):
                        nb0 = (s * SB + j) * 128
                        nc.tensor.matmul(zpA[:, j - j0, :],
                                         lhsT=ftsb[:, nb0:nb0 + 128],
                                         rhs=w1sb[:], start=True, stop=True)
                    if half == 0:
                        nc.scalar.copy(zb8[:, j0:j0 + jn, :], zpA[:, 0:jn, :])
                    else:
                        nc.vector.tensor_copy(zb8[:, j0:j0 + jn, :], zpA[:, 0:jn, :])
                nc.sync.dma_start(
                    out=t1_full[s * SB * 128:s * SB * 128 + nj * 128, :],
                    in_=zb8[:, 0:nj, :])

        # ---------------- edge phase helper ----------------
        def edge_layer(layer):
            R = R1 if layer == 1 else R2
            D = D1 if layer == 1 else D2
            XC = X1C if layer == 1 else X2C
            tabl = table1 if layer == 1 else table2
            srcc = srcc1 if layer == 1 else srcc2
            el_off = (D + 1) * H          # 99 or 195
            er_off = el_off + 3
            if layer == 2:
                pa = ps.tile([128, N_GRAPHS], dt.float32, tag="poolA", name="poolA")
                pb = ps.tile([64, N_GRAPHS], dt.float32, tag="poolB", name="poolB")
            for w in range(NWIN):
                n0 = w * 128
                nw = min(128, NPC - n0)
                cw = cpws[w]
                g0 = offs[w]
                ecols = cw * 128
                # er for this window's nodes (from own shard rows)
                er_w = sb.tile([128, 4], dt.bfloat16, tag="erw", name="erw")
                if layer == 1:
                    if nw < 128:
                        nc.vector.memset(er_w[:, :], 0.0)
                    nc.gpsimd.indirect_dma_start(
                        out=er_w[:, 0:3], out_offset=None, in_=tabl,
                        in_offset=bass.IndirectOffsetOnAxis(
                            ap=dstw1[:, w:w + 1], axis=0),
                        element_offset=er_off)
                else:
                    if nw < 128:
                        nc.vector.memset(er_w[:, :], 0.0)
                    nc.sync.dma_start(out=er_w[:nw, :3],
                                      in_=t2_shard[n0:n0 + nw, er_off:er_off + 3])
                # maskT for er expand: [node, edge]
                dstb_w = sb.tile([128, CPWM * 128], dt.bfloat16, tag="dstb", name="dstb")
                nc.sync.dma_start(out=dstb_w[:, :ecols],
                                  in_=dstb_in[:, g0 * 128:(g0 + cw) * 128])
                maskT = sb.tile([128, CPWM * 128], dt.bfloat16, tag="maskT", name="maskT")
                nc.vector.tensor_scalar(out=maskT[:, :ecols], in0=dstb_w[:, :ecols],
                                        scalar1=iota_col[:, 0:1], scalar2=None,
                                        op0=mybir.AluOpType.is_equal)
                erp = ps.tile([128, CPWM, 4], dt.float32, tag="scratch", name="erp")
                zgs = []
                for c in range(cw):
                    gidx = g0 + c
                    zg = gp.tile([128, R], dt.bfloat16, tag="zg", name="zg")
                    nc.gpsimd.indirect_dma_start(
                        out=zg[:, :], out_offset=None, in_=tabl,
                        in_offset=bass.IndirectOffsetOnAxis(
                            ap=srcc[:, gidx:gidx + 1], axis=0))
                    zgs.append(zg)
                    nc.tensor.matmul(erp[:, c, :3],
                                     lhsT=maskT[:, c * 128:(c + 1) * 128],
                                     rhs=er_w[:, :3], start=True, stop=True)
                # e = el + er ; w = exp(lrelu(e))
                ebuf = sb.tile([128, CPWM, 4], dt.float32, tag="ebuf", name="ebuf")
                for c in range(cw):
                    nc.vector.tensor_tensor(out=ebuf[:, c, :3],
                                            in0=erp[:, c, :3],
                                            in1=zgs[c][:, el_off:el_off + 3],
                                            op=mybir.AluOpType.add)
                esc = sb.tile([128, CPWM, 4], dt.float32, tag="esc", name="esc")
                nc.vector.tensor_scalar(out=esc[:, :cw, :], in0=ebuf[:, :cw, :],
                                        scalar1=NEG, scalar2=None,
                                        op0=mybir.AluOpType.mult)
                elr = sb.tile([128, CPWM, 4], dt.float32, tag="elr", name="elr")
                nc.vector.tensor_tensor(out=elr[:, :cw, :], in0=ebuf[:, :cw, :],
                                        in1=esc[:, :cw, :], op=mybir.AluOpType.max)
                wb = sb.tile([128, CPWM, 4], dt.float32, tag="wb", name="wb")
                nc.scalar.activation(wb[:, :cw, :], elr[:, :cw, :],
                                     mybir.ActivationFunctionType.Exp)
                # weighted masks + aggregation
                aggs = [ps.tile([128, D + 1], dt.float32, tag=f"agg{h}",
                                name=f"agg{h}") for h in range(H)]
                for c in range(cw):
                    gidx = g0 + c
                    wm = sb.tile([128, H, 128], dt.bfloat16, tag="wm", name="wm")
                    for h in range(H):
                        nc.vector.tensor_scalar(
                            out=wm[:, h, :], in0=iota_row[:],
                            scalar1=drelc[:, gidx:gidx + 1],
                            scalar2=wb[:, c, h:h + 1],
                            op0=mybir.AluOpType.is_equal,
                            op1=mybir.AluOpType.mult)
                    for h in range(H):
                        nc.tensor.matmul(
                            out=aggs[h][:, :], lhsT=wm[:, h, :],
                            rhs=zgs[c][:, (D + 1) * h:(D + 1) * (h + 1)],
                            start=(c == 0), stop=(c == cw - 1))
                # epilogue: x = relu(agg_z / s) (* 1/cnt for layer 2)
                seps = sb.tile([128, H], dt.float32, tag="seps", name="seps")
                for h in range(H):
                    nc.vector.tensor_scalar(out=seps[:, h:h + 1],
                                            in0=aggs[h][:, D:D + 1],
                                            scalar1=1e-16, scalar2=None,
                                            op0=mybir.AluOpType.add)
                invs = sb.tile([128, H], dt.float32, tag="invs", name="invs")
                nc.vector.reciprocal(invs[:, :], seps[:, :])
                if layer == 2:
                    nc.vector.tensor_scalar(out=invs[:, :], in0=invs[:, :],
                                            scalar1=invc[:, w:w + 1], scalar2=None,
                                            op0=mybir.AluOpType.mult)
                xdt = dt.float32 if layer == 1 else dt.bfloat16
                xsb = sb.tile([128, XC], xdt, tag="xsb", name="xsb")
                for h in range(H):
                    nc.scalar.activation(xsb[:, h * D:(h + 1) * D],
                                         aggs[h][:, 0:D],
                                         mybir.ActivationFunctionType.Relu,
                                         scale=invs[:, h:h + 1])
                if layer == 1:
                    # transpose x1 -> build zcat2 rows -> t2 shard
                    xtp = ps.tile([X1C, 128], dt.float32, tag="scratch", name="xtp")
                    nc.tensor.transpose(xtp[:, :], xsb[:, :], ident[:])
                    xta = sb.tile([X1C + 1, 128], dt.float32, tag="xta", name="xta")
                    nc.vector.tensor_copy(xta[:X1C, :], xtp[:, :])
                    nc.vector.memset(xta[X1C:, :], 1.0)
                    z2p = ps2.tile([128, R2], dt.float32, tag="zcp", name="z2p")
                    nc.tensor.matmul(z2p[:, :], lhsT=xta[:, :], rhs=w2sb[:],
                                     start=True, stop=True)
                    z2b = sb.tile([128, R2], dt.bfloat16, tag="z2b", name="z2b")
                    nc.vector.tensor_copy(z2b[:, :], z2p[:, :])
                    nc.sync.dma_start(out=t2_shard[n0:n0 + nw, :], in_=z2b[:nw, :])
                else:
                    # pooling: accumulate poolT += x2n^T-blocks @ poolmask
                    pm = sb.tile([128, N_GRAPHS], dt.bfloat16, tag="pm", name="pm")
                    nc.vector.tensor_scalar(out=pm[:], in0=iota_row[:],
                                            scalar1=gidc[:, w:w + 1], scalar2=None,
                                            op0=mybir.AluOpType.is_equal)
                    nc.tensor.matmul(pa[:, :], lhsT=xsb[:, 0:128], rhs=pm[:],
                                     start=(w == 0), stop=(w == NWIN - 1))
                    nc.tensor.matmul(pb[:, :], lhsT=xsb[:, 128:192], rhs=pm[:],
                                     start=(w == 0), stop=(w == NWIN - 1))
            if layer == 2:
                return pa, pb

        edge_layer(1)
        nc.gpsimd.collective_compute(
            "AllGather", mybir.AluOpType.bypass, replica_groups=rg,
            ins=[t2_shard[:, :]], outs=[table2_t[:, :]])
        nc.sync.dma_start(out=table2, in_=table2_t[:, :])

        pa, pb = edge_layer(2)

        # ---------------- pooling reduce + head ----------------
        pasb = sb.tile([128, N_GRAPHS], dt.float32, tag="pasb", name="pasb")
        nc.vector.tensor_copy(pasb[:, :], pa[:, :])
        pbsb = sb.tile([64, N_GRAPHS], dt.float32, tag="pbsb", name="pbsb")
        nc.vector.tensor_copy(pbsb[:, :], pb[:, :])
        nc.sync.dma_start(out=pool_loc[0:128, :], in_=pasb[:, :])
        nc.sync.dma_start(out=pool_loc[128:192, :], in_=pbsb[:, :])
        pra = sb.tile([128, N_GRAPHS], dt.float32, tag="pra", name="pra")
        prb = sb.tile([65, N_GRAPHS], dt.float32, tag="prb", name="prb")
        nc.gpsimd.collective_compute(
            "AllReduce", mybir.AluOpType.add, replica_groups=rg,
            ins=[pool_loc[:, :]], outs=[pool_red])
        nc.sync.dma_start(out=pra[:, :], in_=pool_red_t[0:128, :])
        nc.sync.dma_start(out=prb[:64, :], in_=pool_red_t[128:192, :])
        nc.vector.memset(prb[64:, :], 1.0)

        u1 = ps.tile([64, N_GRAPHS], dt.float32, tag="poolA", name="u1")
        nc.tensor.matmul(u1[:, :], lhsT=d1a[:, :], rhs=pra[:, :], start=True, stop=False)
        nc.tensor.matmul(u1[:, :], lhsT=d1b[:, :], rhs=prb[:, :], start=False, stop=True)
        h1 = sb.tile([65, N_GRAPHS], dt.float32, tag="h1", name="h1")
        nc.scalar.activation(h1[:64, :], u1[:, :], mybir.ActivationFunctionType.Relu)
        nc.vector.memset(h1[64:, :], 1.0)
        o_ps = ps.tile([N_GRAPHS, 1], dt.float32, tag="poolB", name="ops")
        nc.tensor.matmul(o_ps[:, :], lhsT=h1[:, :], rhs=d2w[:, :], start=True, stop=True)
        osb = sb.tile([N_GRAPHS, 1], dt.float32, tag="osb", name="osb")
        nc.vector.tensor_copy(osb[:, :], o_ps[:, :])
        nc.sync.dma_start(out=out_ext, in_=osb[:, :])

    nc.finalize()
    return nc


# ======================= host side =======================

def _rho():
    """Table-1 row permutation: node n -> DRAM row written by phase A."""
    n = np.arange(N1PAD)
    s = n // (SB * 128)
    rem = n % (SB * 128)
    j = rem // 128
    p = rem % 128
    nj = np.where(s < NB1 // SB, SB, NB1 % SB)
    return (s * (SB * 128) + p * nj + j).astype(np.int32)


def _prep(feature, src, dst, graph_ids, W1, al1, ar1, W2, al2, ar2,
          d1_w, d1_b, d2_w, d2_b):
    feature = np.asarray(feature, np.float32)
    src = np.asarray(src, np.int64)
    dst = np.asarray(dst, np.int64)
    graph_ids = np.asarray(graph_ids, np.int64)

    order = np.argsort(dst, kind="stable")
    src_s = src[order].astype(np.int32)
    dst_s = dst[order].astype(np.int32)

    # edge ranges per node window
    cnts = np.bincount(graph_ids, minlength=N_GRAPHS).astype(np.float32)
    cnts = np.maximum(cnts, 1.0)
    node_inv = (1.0 / cnts)[graph_ids]            # per node 1/cnt

    # window boundaries; per-window chunk counts = max over cores
    percore = []
    cpws = [1] * NWIN
    for r in range(NC):
        wins = []
        for w in range(NWIN):
            lo = r * NPC + w * 128
            hi = min(r * NPC + NPC, lo + 128)
            e0 = np.searchsorted(dst_s, lo, side="left")
            e1 = np.searchsorted(dst_s, hi, side="left")
            wins.append((lo, hi, e0, e1))
            cpws[w] = max(cpws[w], _ceil(max(e1 - e0, 1), 128))
        percore.append(wins)

    cpws = tuple(cpws)
    offs = [0]
    for c_ in cpws:
        offs.append(offs[-1] + c_)
    TCH = offs[-1]

    rho = _rho()
    # weight prep
    W1 = np.asarray(W1, np.float32); W2 = np.asarray(W2, np.float32)
    al1 = np.asarray(al1, np.float32); ar1 = np.asarray(ar1, np.float32)
    al2 = np.asarray(al2, np.float32); ar2 = np.asarray(ar2, np.float32)

    def wcat(W, al, ar, D, R):
        F = W.shape[0]
        A_l = np.zeros((H * D, H), np.float32)
        A_r = np.zeros((H * D, H), np.float32)
        for h in range(H):
            A_l[h * D:(h + 1) * D, h] = al[h]
            A_r[h * D:(h + 1) * D, h] = ar[h]
        Wl = W @ A_l   # [F, 3]
        Wr = W @ A_r
        out = np.zeros((F + 1, R), np.float32)
        for h in range(H):
            out[:F, h * (D + 1):h * (D + 1) + D] = W[:, h * D:(h + 1) * D]
            out[F, h * (D + 1) + D] = 1.0          # ones column
        el_off = (D + 1) * H
        out[:F, el_off:el_off + 3] = Wl
        out[:F, el_off + 3:el_off + 6] = Wr
        return out

    wcat1 = wcat(W1, al1, ar1, D1, R1).astype(ml_dtypes.bfloat16)
    wcat2 = wcat(W2, al2, ar2, D2, R2)

    d1_w = np.asarray(d1_w, np.float32); d1_b = np.asarray(d1_b, np.float32)
    d2_w = np.asarray(d2_w, np.float32); d2_b = np.asarray(d2_b, np.float32)
    d1a = d1_w[0:128, :].copy()
    d1b = np.vstack([d1_w[128:192, :], d1_b[None, :]]).astype(np.float32)
    d2a = np.vstack([d2_w, d2_b[None, :]]).astype(np.float32)

    featTb = np.zeros((11, N1PAD), np.float32)
    featTb[:10, :N_NODES] = feature.T
    featTb[10, :N_NODES] = 1.0
    featTb = featTb.astype(ml_dtypes.bfloat16)

    in_maps = []
    for r in range(NC):
        srcc = np.zeros((128, TCH), np.int32)
        drel = np.full((128, TCH), -1.0, np.float32)
        for w, (lo, hi, e0, e1) in enumerate(percore[r]):
            cnt = e1 - e0
            nch = cpws[w]
            s = np.zeros(nch * 128, np.int32)
            d = np.full(nch * 128, -1.0, np.float32)
            s[:cnt] = src_s[e0:e1]
            d[:cnt] = (dst_s[e0:e1] - lo).astype(np.float32)
            srcc[:, offs[w]:offs[w] + nch] = s.reshape(nch, 128).T
            drel[:, offs[w]:offs[w] + nch] = d.reshape(nch, 128).T
        dstb = np.broadcast_to(
            drel.T.reshape(1, TCH * 128), (128, TCH * 128)
        ).astype(ml_dtypes.bfloat16)
        # NOTE: drel.T.reshape gives edge j=(chunk, p) flattened chunk-major:
        # dstb[:, chunk*128 + p] = drel[p, chunk] -- matches maskT slicing.
        gidc = np.full((128, NWIN), -1.0, np.float32)
        invc = np.zeros((128, NWIN), np.float32)
        dstw1 = np.zeros((128, NWIN), np.int32)
        for w in range(NWIN):
            lo, hi, _, _ = percore[r][w]
            nw = hi - lo
            gidc[:nw, w] = graph_ids[lo:hi].astype(np.float32)
            invc[:nw, w] = node_inv[lo:hi]
            dstw1[:nw, w] = rho[np.arange(lo, hi)]
        in_maps.append({
            "featTb": featTb, "wcat1": wcat1, "wcat2": wcat2,
            "srcc1": rho[srcc], "srcc2": srcc, "drelc": drel,
            "dstb": np.ascontiguousarray(dstb),
            "gidc": gidc, "invc": invc, "dstw1": dstw1,
            "d1a": d1a, "d1b": d1b, "d2": d2a,
        })
    return in_maps, cpws


def kernel(**inputs):
    in_maps, cpws = _prep(**inputs)
    key = cpws
    if key not in _CACHE:
        _CACHE[key] = build_program(cpws)
    nc = _CACHE[key]
    res = run_bass_kernel_spmd(nc, in_maps, list(range(NC)))
    return res.results[0]["out"]

